# revision 1
# baseline (speedup 1.0000x reference)
"""Distributed Bass kernel for nn_LACF (gnn_message_passing) on 8 TRN2 cores.

Strategy: shard nodes (and their incoming edges, since segment_sum is over
h_idx) across 8 cores. Each core owns R=N/8 node rows. Edges are bucketed by
(core, 128-node block) on the host; each block's edges are padded to T tiles
of 128 edges so every core runs an identical static program.

Per layer:
  node phase (local shard): update tables from previous segment sums, compute
    A1=e1@W1top+b1, B1=e1@W1bot, x2=sigmoid(gumbel+MLP(e2))*e2; write a packed
    bf16 row table [e0|x2|e1|B1] (512B/row); AllGather -> full packed table.
  edge phase: one indirect 512B-row gather per edge from the packed table,
    one 128B-row gather of A1[h] (local), edge MLP on gathered tiles, and
    segment-sum via PSUM-accumulated matmuls with host-built one-hot P
    matrices carrying max(G,eps) values. Branch1 reweighting uses
    wG = sigmoid(...)/max(G,eps) as an rhs scale so all three branches share
    one lhsT per tile. Row sums ride along as a 65th rhs column.
"""

import sys

if "/opt/trn_rl_repo" not in sys.path:
    sys.path.insert(0, "/opt/trn_rl_repo")

import numpy as np
import ml_dtypes

BF16 = ml_dtypes.bfloat16
G_EPS = np.float32(1e-6)
ROW_EPS = 1e-30


def _prep(inputs, ncores):
    """Host-side sharding: bucket edges by (core, node-block), build index
    tiles, gumbel/recipG columns and valued one-hot P tiles."""
    h = np.asarray(inputs["h_idx"]).astype(np.int64).ravel()
    t = np.asarray(inputs["t_idx"]).astype(np.int64).ravel()
    G = np.asarray(inputs["G_values"]).astype(np.float32).ravel()
    eg = np.asarray(inputs["edge_gumbel"]).astype(np.float32)
    emb0 = np.asarray(inputs["emb0"]).astype(np.float32)
    ngum = np.asarray(inputs["emb_gumbel"]).astype(np.float32)

    N, D = emb0.shape
    E = h.shape[0]
    L = eg.shape[0]
    assert N % ncores == 0
    RS = N // ncores                      # real rows per core
    nb = (RS + 127) // 128                # node blocks per core
    R = nb * 128                          # padded rows per core

    core_of = h // RS
    hloc = h - core_of * RS
    blk = hloc // 128
    key = (core_of * nb + blk).astype(np.int64)
    order = np.argsort(key, kind="stable")
    counts = np.bincount(key, minlength=ncores * nb)
    T = max(1, int(-(-counts.max() // 128)))
    ET = nb * T

    starts = np.zeros(ncores * nb, np.int64)
    starts[1:] = np.cumsum(counts)[:-1]
    sk = key[order]
    rank = np.arange(E) - starts[sk]
    j = (rank // 128).astype(np.int64)
    p = (rank % 128).astype(np.int64)
    c = core_of[order]
    b = blk[order]
    col = b * T + j

    tso = t[order]
    tgid = (tso // RS) * R + (tso - (tso // RS) * RS)  # padded global row id

    tid = np.zeros((ncores, 128, ET), np.int32)
    hid = np.zeros((ncores, 128, ET), np.int32)
    egc = np.zeros((ncores, L, 128, ET), np.float32)
    rg = np.zeros((ncores, 128, ET), np.float32)
    p0 = np.zeros((ncores, nb, 128, T * 128), BF16)

    tid[c, p, col] = tgid.astype(np.int32)
    hid[c, p, col] = hloc[order].astype(np.int32)
    egc[c, :, p, col] = eg[:, order].T
    gsafe = np.maximum(G[order], G_EPS)
    rg[c, p, col] = (np.float32(1.0) / gsafe)
    noff = (hloc[order] % 128).astype(np.int64)
    p0[c, b, p, j * 128 + noff] = gsafe.astype(BF16)

    # node-sharded tensors
    embc = np.zeros((ncores, R, D), np.float32)
    gumc = np.zeros((ncores, L, R, D), np.float32)
    for cc in range(ncores):
        embc[cc, :RS] = emb0[cc * RS:(cc + 1) * RS]
        gumc[cc, :, :RS] = ngum[:, cc * RS:(cc + 1) * RS]

    return dict(N=N, D=D, E=E, L=L, RS=RS, nb=nb, R=R, T=T, ET=ET,
                tid=tid, hid=hid, egc=egc, rg=rg, p0=p0, embc=embc, gumc=gumc)


def build_program(cfg):
    import concourse.bacc as bacc
    import concourse.mybir as mybir
    import concourse.tile as tile
    from concourse.masks import make_identity

    nb, T, L, NCC = cfg["nb"], cfg["T"], cfg["L"], cfg["ncores"]
    D = cfg["D"]
    R = nb * 128
    NF = NCC * R
    ET = nb * T
    PK = 4 * D                     # packed row elems
    b2v = cfg["b2"]                # per-layer python floats
    inv_t = cfg["inv_t"]

    f32 = mybir.dt.float32
    bf = mybir.dt.bfloat16
    i32 = mybir.dt.int32

    nc = bacc.Bacc("TRN2", target_bir_lowering=False)

    P_in = {}
    for name, shape, dt in [
        ("emb", [R, D], f32), ("gum", [L, R, D], f32),
        ("p0", [nb, 128, T * 128], bf), ("tidx", [128, ET], i32),
        ("hidx", [128, ET], i32), ("egum", [L, 128, ET], f32),
        ("rgs", [128, ET], f32),
        ("w1t", [L, D, D], f32), ("w1b", [L, D, D], f32), ("b1", [L, D], f32),
        ("w2", [L, 128, T * D], f32),
        ("ew1", [L, D, D], f32), ("ew2", [L, D, D], f32),
        ("eb1", [L, D], f32), ("eb2", [L, D], f32),
    ]:
        P_in[name] = nc.dram_tensor(name, shape, dt, kind="ExternalInput")
    out = nc.dram_tensor("out", [3, R, D], f32, kind="ExternalOutput")

    rg_all = [list(range(NCC))]

    with tile.TileContext(nc) as tc:
        with (
            tc.tile_pool(name="dram", bufs=1, space="DRAM") as dram,
            tc.tile_pool(name="const", bufs=1) as constp,
            tc.tile_pool(name="nodew", bufs=3) as nodew,
            tc.tile_pool(name="chunkw", bufs=2) as chunkw,
            tc.tile_pool(name="edgew", bufs=3) as edgew,
            tc.tile_pool(name="ps", bufs=2, space="PSUM") as psp,
            tc.tile_pool(name="psacc", bufs=2, space="PSUM") as psaccp,
        ):
            # ---- persistent DRAM state
            e0d = dram.tile([R, D], f32, name="e0d")
            e1d = dram.tile([R, D], f32, name="e1d")
            e2d = dram.tile([R, D], f32, name="e2d")
            s0d = dram.tile([R, D], f32, name="s0d")
            s1d = dram.tile([R, D], f32, name="s1d")
            s2d = dram.tile([R, D], f32, name="s2d")
            a1d = dram.tile([R, D], bf, name="a1d")
            gnnd = dram.tile([R, 193], f32, name="gnnd")
            pshard = dram.tile([R, PK], bf, name="pshard")
            pfull = [dram.tile([NF, PK], bf, name=f"pfull{i}",
                               addr_space="Shared") for i in range(L)]

            # ---- constants resident in SBUF
            ident = constp.tile([128, 128], f32, name="ident")
            make_identity(nc, ident[:])
            tsb = constp.tile([128, ET], i32, name="tsb")
            nc.sync.dma_start(out=tsb[:], in_=P_in["tidx"][:, :])
            hsb = constp.tile([128, ET], i32, name="hsb")
            nc.sync.dma_start(out=hsb[:], in_=P_in["hidx"][:, :])
            rgsb = constp.tile([128, ET], f32, name="rgsb")
            nc.sync.dma_start(out=rgsb[:], in_=P_in["rgs"][:, :])
            egsb = [constp.tile([128, ET], f32, name=f"egsb{i}") for i in range(L)]
            for i in range(L):
                nc.sync.dma_start(out=egsb[i][:], in_=P_in["egum"][i, :, :])
            w2sb = [constp.tile([128, T, D], f32, name=f"w2sb{i}")
                    for i in range(L)]
            for i in range(L):
                nc.sync.dma_start(out=w2sb[i][:], in_=P_in["w2"][i, :, :])
            wt = {}
            for wname in ("w1t", "w1b", "ew1", "ew2"):
                for i in range(L):
                    wtile = constp.tile([D, D], f32, name=f"{wname}{i}")
                    nc.sync.dma_start(out=wtile[:], in_=P_in[wname][i, :, :])
                    wt[(wname, i)] = wtile
            for bname in ("b1", "eb1", "eb2"):
                for i in range(L):
                    btile = constp.tile([D, 1], f32, name=f"{bname}{i}")
                    nc.sync.dma_start(out=btile[:], in_=P_in[bname][i, :, None])
                    wt[(bname, i)] = btile

            # ---- prologue: init tables from emb
            for dst in (e0d, e1d, e2d, s0d, s1d, s2d):
                nc.sync.dma_start(out=dst[:], in_=P_in["emb"][:, :])

            Relu = mybir.ActivationFunctionType.Relu
            Sigm = mybir.ActivationFunctionType.Sigmoid
            Ident = mybir.ActivationFunctionType.Identity
            Copy = mybir.ActivationFunctionType.Copy
            AX = mybir.AxisListType.X
            ADD = mybir.AluOpType.add
            MUL = mybir.AluOpType.mult

            def update_tiles(r0, cs, write_out=False):
                """Apply e += gnn (branch1 scaled by dinv), s += e for rows
                [r0, r0+cs*128). Returns updated (e0t, e1t, e2t) SBUF tiles."""
                rows = slice(r0, r0 + cs * 128)
                gt = nodew.tile([128, cs, 193], f32, tag="gt")
                nc.sync.dma_start(
                    out=gt[:], in_=gnnd[rows].rearrange("(c p) d -> p c d", p=128))
                ets = []
                for kname, kd in (("e0", e0d), ("e1", e1d), ("e2", e2d)):
                    et = nodew.tile([128, cs, D], f32, tag=f"{kname}t")
                    nc.sync.dma_start(
                        out=et[:], in_=kd[rows].rearrange("(c p) d -> p c d", p=128))
                    ets.append(et)
                e0t, e1t, e2t = ets
                # branch1: dinv per node row of each sub-tile
                for q in range(cs):
                    row = gt[:, q, 192:193]
                    rsafe = nodew.tile([128, 1], f32, tag="rsafe")
                    nc.vector.tensor_scalar_max(out=rsafe[:], in0=row, scalar1=ROW_EPS)
                    dinv = nodew.tile([128, 1], f32, tag="dinv")
                    nc.vector.reciprocal(out=dinv[:], in_=rsafe[:])
                    g1s = nodew.tile([128, D], f32, tag="g1s")
                    nc.vector.tensor_scalar_mul(
                        out=g1s[:], in0=gt[:, q, 128:192], scalar1=dinv[:, 0:1])
                    nc.vector.tensor_add(
                        out=e1t[:, q, :], in0=e1t[:, q, :], in1=g1s[:])
                nc.vector.tensor_add(out=e0t[:], in0=e0t[:], in1=gt[:, :, 0:64])
                nc.vector.tensor_add(out=e2t[:], in0=e2t[:], in1=gt[:, :, 64:128])
                for kd, et in ((e0d, e0t), (e1d, e1t), (e2d, e2t)):
                    nc.sync.dma_start(
                        out=kd[rows].rearrange("(c p) d -> p c d", p=128), in_=et[:])
                for kname, sd, et in (("s0", s0d, e0t), ("s1", s1d, e1t),
                                      ("s2", s2d, e2t)):
                    stl = nodew.tile([128, cs, D], f32, tag=f"{kname}t")
                    nc.sync.dma_start(
                        out=stl[:], in_=sd[rows].rearrange("(c p) d -> p c d", p=128))
                    nc.vector.tensor_add(out=stl[:], in0=stl[:], in1=et[:])
                    nc.sync.dma_start(
                        out=sd[rows].rearrange("(c p) d -> p c d", p=128), in_=stl[:])
                    if write_out:
                        k = int(kname[1])
                        nc.sync.dma_start(
                            out=out[k, rows].rearrange("(c p) d -> p c d", p=128),
                            in_=stl[:])
                return e0t, e1t, e2t

            def node_phase(i):
                """Update (i>0), compute A1/B1/x2, pack, AllGather."""
                n_chunks = -(-nb // 4)
                for ch in range(n_chunks):
                    b0 = ch * 4
                    cs = min(4, nb - b0)
                    r0 = b0 * 128
                    rows = slice(r0, r0 + cs * 128)
                    CF = cs * 128
                    if i > 0:
                        e0t, e1t, e2t = update_tiles(r0, cs)
                    else:
                        e0t = nodew.tile([128, cs, D], f32, tag="e0t")
                        e1t = nodew.tile([128, cs, D], f32, tag="e1t")
                        e2t = nodew.tile([128, cs, D], f32, tag="e2t")
                        for et, kd in ((e0t, e0d), (e1t, e1d), (e2t, e2d)):
                            nc.sync.dma_start(
                                out=et[:],
                                in_=kd[rows].rearrange("(c p) d -> p c d", p=128))
                    # transpose e1,e2 sub-tiles -> feat-major chunk [64, CF]
                    e1T = chunkw.tile([D, CF], f32, tag="e1T")
                    e2T = chunkw.tile([D, CF], f32, tag="e2T")
                    for q in range(cs):
                        for src, dstT in ((e1t, e1T), (e2t, e2T)):
                            pt = psp.tile([D, 128], f32, tag="ptr")
                            nc.tensor.transpose(
                                out=pt[:], in_=src[:, q, :], identity=ident[:])
                            nc.scalar.activation(
                                out=dstT[:, q * 128:(q + 1) * 128], in_=pt[:], func=Copy)
                    # feat-major matmuls
                    a1T = chunkw.tile([D, CF], f32, tag="a1T")
                    b1T = chunkw.tile([D, CF], f32, tag="b1T")
                    lgT = chunkw.tile([D, CF], f32, tag="lgT")
                    pm = psp.tile([D, CF], f32, tag="pmm")
                    nc.tensor.matmul(out=pm[:], lhsT=wt[("w1t", i)][:], rhs=e1T[:],
                                     start=True, stop=True)
                    nc.scalar.activation(out=a1T[:], in_=pm[:], func=Ident,
                                         bias=wt[("b1", i)][:, 0:1])
                    pm2 = psp.tile([D, CF], f32, tag="pmm")
                    nc.tensor.matmul(out=pm2[:], lhsT=wt[("w1b", i)][:], rhs=e1T[:],
                                     start=True, stop=True)
                    nc.scalar.activation(out=b1T[:], in_=pm2[:], func=Copy)
                    pm3 = psp.tile([D, CF], f32, tag="pmm")
                    nc.tensor.matmul(out=pm3[:], lhsT=wt[("ew1", i)][:], rhs=e2T[:],
                                     start=True, stop=True)
                    hidT = chunkw.tile([D, CF], f32, tag="hidT")
                    nc.scalar.activation(out=hidT[:], in_=pm3[:], func=Relu,
                                         bias=wt[("eb1", i)][:, 0:1])
                    pm4 = psp.tile([D, CF], f32, tag="pmm")
                    nc.tensor.matmul(out=pm4[:], lhsT=wt[("ew2", i)][:], rhs=hidT[:],
                                     start=True, stop=True)
                    nc.scalar.activation(out=lgT[:], in_=pm4[:], func=Ident,
                                         bias=wt[("eb2", i)][:, 0:1])
                    # back to node-major, assemble packed tiles + A1
                    pk = nodew.tile([128, cs, PK], bf, tag="pk")
                    a1bf = nodew.tile([128, cs, D], bf, tag="a1bf")
                    for q in range(cs):
                        cols = slice(q * 128, (q + 1) * 128)
                        pa = psp.tile([128, D], f32, tag="ptr")
                        nc.tensor.transpose(out=pa[:], in_=a1T[:, cols],
                                            identity=ident[0:64, 0:64])
                        nc.vector.tensor_copy(out=a1bf[:, q, :], in_=pa[:])
                        pb = psp.tile([128, D], f32, tag="ptr")
                        nc.tensor.transpose(out=pb[:], in_=b1T[:, cols],
                                            identity=ident[0:64, 0:64])
                        nc.vector.tensor_copy(out=pk[:, q, 192:256], in_=pb[:])
                        pl = psp.tile([128, D], f32, tag="ptr")
                        nc.tensor.transpose(out=pl[:], in_=lgT[:, cols],
                                            identity=ident[0:64, 0:64])
                        gmt = nodew.tile([128, D], f32, tag="gmt")
                        nc.sync.dma_start(out=gmt[:],
                                          in_=P_in["gum"][i, r0 + q * 128:
                                                          r0 + (q + 1) * 128, :])
                        lgn = nodew.tile([128, D], f32, tag="lgn")
                        nc.vector.tensor_add(out=lgn[:], in0=pl[:], in1=gmt[:])
                        gate = nodew.tile([128, D], f32, tag="gate")
                        nc.scalar.activation(out=gate[:], in_=lgn[:], func=Sigm,
                                             scale=inv_t)
                        nc.vector.tensor_mul(out=pk[:, q, 64:128], in0=gate[:],
                                             in1=e2t[:, q, :])
                        nc.vector.tensor_copy(out=pk[:, q, 0:64], in_=e0t[:, q, :])
                        nc.vector.tensor_copy(out=pk[:, q, 128:192], in_=e1t[:, q, :])
                    nc.sync.dma_start(
                        out=a1d[rows].rearrange("(c p) d -> p c d", p=128),
                        in_=a1bf[:])
                    nc.sync.dma_start(
                        out=pshard[rows].rearrange("(c p) d -> p c d", p=128),
                        in_=pk[:])
                nc.gpsimd.collective_compute(
                    "AllGather", mybir.AluOpType.bypass, replica_groups=rg_all,
                    ins=[pshard.opt()], outs=[pfull[i].opt()])

            def edge_phase(i):
                import concourse.bass as bass
                for b in range(nb):
                    ecols = slice(b * T, (b + 1) * T)
                    p0blk = edgew.tile([128, T * 128], bf, tag="p0blk")
                    nc.sync.dma_start(out=p0blk[:], in_=P_in["p0"][b, :, :])
                    gt = edgew.tile([128, T, PK], bf, tag="gtile")
                    at = edgew.tile([128, T, D], bf, tag="atile")
                    for jj in range(T):
                        ec = b * T + jj
                        nc.gpsimd.indirect_dma_start(
                            out=gt[:, jj, :], out_offset=None, in_=pfull[i][:],
                            in_offset=bass.IndirectOffsetOnAxis(
                                ap=tsb[:, ec:ec + 1], axis=0))
                        nc.gpsimd.indirect_dma_start(
                            out=at[:, jj, :], out_offset=None, in_=a1d[:],
                            in_offset=bass.IndirectOffsetOnAxis(
                                ap=hsb[:, ec:ec + 1], axis=0))
                    # edge MLP -> w
                    pre = edgew.tile([128, T, D], bf, tag="pre")
                    nc.vector.tensor_tensor(out=pre[:], in0=at[:],
                                            in1=gt[:, :, 192:256], op=ADD)
                    rel = edgew.tile([128, T, D], bf, tag="rel")
                    nc.scalar.activation(out=rel[:], in_=pre[:], func=Relu)
                    mr = edgew.tile([128, T, D], f32, tag="mr")
                    nc.vector.tensor_tensor(
                        out=mr[:], in0=rel[:],
                        in1=w2sb[i][:], op=MUL)
                    lg = edgew.tile([128, T], f32, tag="lgE")
                    nc.vector.tensor_reduce(out=lg[:], in_=mr[:], axis=AX, op=ADD)
                    lg2 = edgew.tile([128, T], f32, tag="lg2E")
                    nc.vector.tensor_add(out=lg2[:], in0=lg[:], in1=egsb[i][:, ecols])
                    wv = edgew.tile([128, T], f32, tag="wv")
                    nc.scalar.activation(out=wv[:], in_=lg2[:], func=Sigm,
                                         scale=inv_t, bias=float(b2v[i]) * inv_t)
                    wg = edgew.tile([128, T], f32, tag="wg")
                    nc.vector.tensor_mul(out=wg[:], in0=wv[:], in1=rgsb[:, ecols])
                    # segment-sum matmuls
                    pacc02 = psaccp.tile([128, 128], f32, tag="pacc02")
                    pacc1 = psaccp.tile([128, 65], f32, tag="pacc1")
                    for jj in range(T):
                        st = edgew.tile([128, 65], bf, tag="st")
                        nc.vector.tensor_scalar_mul(
                            out=st[:, 0:64], in0=gt[:, jj, 128:192],
                            scalar1=wg[:, jj:jj + 1])
                        nc.vector.tensor_copy(out=st[:, 64:65], in_=wg[:, jj:jj + 1])
                        lhs = p0blk[:, jj * 128:(jj + 1) * 128]
                        nc.tensor.matmul(out=pacc02[:], lhsT=lhs,
                                         rhs=gt[:, jj, 0:128],
                                         start=(jj == 0), stop=(jj == T - 1))
                        nc.tensor.matmul(out=pacc1[:], lhsT=lhs, rhs=st[:],
                                         start=(jj == 0), stop=(jj == T - 1))
                    gout = edgew.tile([128, 193], f32, tag="gout")
                    nc.scalar.activation(out=gout[:, 0:128], in_=pacc02[:], func=Copy)
                    nc.scalar.activation(out=gout[:, 128:193], in_=pacc1[:], func=Copy)
                    nc.sync.dma_start(out=gnnd[b * 128:(b + 1) * 128, :], in_=gout[:])

            for i in range(L):
                node_phase(i)
                edge_phase(i)
            # final update + output
            n_chunks = -(-nb // 4)
            for ch in range(n_chunks):
                b0 = ch * 4
                cs = min(4, nb - b0)
                update_tiles(b0 * 128, cs, write_out=True)

    if not nc.is_finalized():
        nc.finalize()
    return nc


def _setup(inputs, ncores=8):
    """Host prep + program build + per-core input maps."""
    pc = _prep(inputs, ncores)
    D, T = pc["D"], pc["T"]
    eW1 = np.asarray(inputs["edge_W1"]).astype(np.float32)
    eW2 = np.asarray(inputs["edge_W2"]).astype(np.float32)
    cfg = dict(nb=pc["nb"], T=T, L=pc["L"], ncores=ncores, D=D,
               b2=[float(x) for x in np.asarray(inputs["edge_b2"]).ravel()],
               inv_t=1.0)
    nc = build_program(cfg)
    w2t = np.broadcast_to(np.tile(eW2[:, :, 0], (1, T))[:, None, :],
                          (eW2.shape[0], 128, T * eW2.shape[1])).copy()
    shared = {
        "w1t": np.ascontiguousarray(eW1[:, :D, :]),
        "w1b": np.ascontiguousarray(eW1[:, D:, :]),
        "b1": np.asarray(inputs["edge_b1"]).astype(np.float32),
        "w2": w2t,
        "ew1": np.asarray(inputs["emb_W1"]).astype(np.float32),
        "ew2": np.asarray(inputs["emb_W2"]).astype(np.float32),
        "eb1": np.asarray(inputs["emb_b1"]).astype(np.float32),
        "eb2": np.asarray(inputs["emb_b2"]).astype(np.float32),
    }
    in_maps = []
    for c in range(ncores):
        m = {"emb": pc["embc"][c], "gum": pc["gumc"][c],
             "p0": pc["p0"][c], "tidx": pc["tid"][c], "hidx": pc["hid"][c],
             "egum": pc["egc"][c], "rgs": pc["rg"][c]}
        m.update(shared)
        in_maps.append(m)
    return nc, in_maps, pc


def kernel(**inputs) -> np.ndarray:
    from concourse.bass_utils import run_bass_kernel_spmd

    NCC = 8
    nc, in_maps, pc = _setup(inputs, NCC)
    RS, N, D = pc["RS"], pc["N"], pc["D"]
    res = run_bass_kernel_spmd(nc, in_maps, list(range(NCC)))
    full = np.empty((3, N, D), np.float32)
    for c in range(NCC):
        full[:, c * RS:(c + 1) * RS] = res.results[c]["out"][:, :RS]
    return full

